# revision 2
# baseline (speedup 1.0000x reference)
"""Trainium2 Bass kernel for nn_DecomposeModel (gated 2-layer MLP decompose).

Strategy:
  - Host: sort rows by group. Only group==0 rows need the left GateNN,
    only group==1 rows need the right GateNN, group==2 rows output zero.
    Deal g0/g1 rows round-robin across the 8 cores (data parallel), pad
    each segment to a fixed per-core cap so all cores run one SPMD program.
  - Device: activations kept transposed [feature, row] so every matmul
    uses the weights in native [in, out] layout as the stationary operand
    (out = W_tile.T @ xT_tile). bf16 matmuls, f32 PSUM accumulation,
    tanh/sigmoid on ScalarE with fused bias, gating product on VectorE.
  - Host: scatter device outputs back to full [B, H] f32 (g2 rows stay 0).
"""

import sys

try:
    import concourse  # noqa: F401
except ImportError:
    sys.path.insert(0, "/opt/trn_rl_repo")

import numpy as np
import ml_dtypes

import concourse.tile as tile
from concourse import bacc, mybir
from concourse.bass_utils import run_bass_kernel_spmd

B = 32768
H = 512
NCORES = 8
BC = B // NCORES  # per-core shard of the mask output
DEFAULT_CAP = 1472  # per-core per-branch row capacity (B/3/8 = 1365.3 avg)

BF16 = mybir.dt.bfloat16
F32 = mybir.dt.float32
I32 = mybir.dt.int32

_PROGRAM_CACHE = {}
LAST_RESULT = None  # BassKernelResults of the most recent kernel() call


def _blocks(cap):
    out = []
    rem = cap
    while rem > 0:
        b = 512 if rem >= 576 else rem
        out.append(b)
        rem -= b
    return out


def build_program(cap0, cap1):
    """Emit + compile the SPMD program for per-branch caps (cap0, cap1)."""
    nc = bacc.Bacc("TRN2", target_bir_lowering=False, debug=False,
                   num_devices=NCORES)

    ncols = cap0 + cap1
    x1t_d = nc.dram_tensor("x1t", [3 * H, ncols], BF16, kind="ExternalInput")
    lt_d = nc.dram_tensor("lt", [H, cap1], BF16, kind="ExternalInput")
    grp_d = nc.dram_tensor("grp", [BC], I32, kind="ExternalInput")

    w_d = {}
    for name, kdim in [("wl1h", 3 * H), ("wl1g", 3 * H),
                       ("wr1h", 3 * H), ("wr1g", 3 * H),
                       ("wl2h", H), ("wl2g", H),
                       ("wr2h", 2 * H), ("wr2g", 2 * H)]:
        w_d[name] = nc.dram_tensor(name, [kdim, H], BF16, kind="ExternalInput")
    b_d = {}
    for name in ["bl1h", "bl1g", "bl2h", "bl2g",
                 "br1h", "br1g", "br2h", "br2g"]:
        b_d[name] = nc.dram_tensor(name, [H], F32, kind="ExternalInput")

    outt_d = nc.dram_tensor("outt", [H, ncols], F32, kind="ExternalOutput")
    fin_d = nc.dram_tensor("fin", [BC], I32, kind="ExternalOutput")

    with tile.TileContext(nc) as tc:
        with (
            tc.tile_pool(name="wsb", bufs=1) as wpool,
            tc.tile_pool(name="bsb", bufs=1) as bpool,
            tc.tile_pool(name="xsb", bufs=3) as xpool,
            tc.tile_pool(name="hsb", bufs=2) as hpool,
            tc.tile_pool(name="act", bufs=3) as apool,
            tc.tile_pool(name="osb", bufs=2) as opool,
            tc.tile_pool(name="msc", bufs=1) as mpool,
            tc.tile_pool(name="ps", bufs=8, space="PSUM") as pspool,
        ):
            # --- persistent weights / biases -------------------------------
            w_sb = {}
            for name, kdim in [("wl1h", 3 * H), ("wl1g", 3 * H),
                               ("wr1h", 3 * H), ("wr1g", 3 * H),
                               ("wl2h", H), ("wl2g", H),
                               ("wr2h", 2 * H), ("wr2g", 2 * H)]:
                nk = kdim // 128
                t = wpool.tile([128, nk, H], BF16, tag=f"w_{name}")
                nc.sync.dma_start(
                    out=t[:],
                    in_=w_d[name].rearrange("(kt p) f -> p kt f", p=128),
                )
                w_sb[name] = t
            b_sb = {}
            for name in b_d:
                t = bpool.tile([128, 4], F32, tag=f"b_{name}")
                nc.sync.dma_start(
                    out=t[:], in_=b_d[name].rearrange("(j p) -> p j", p=128)
                )
                b_sb[name] = t

            # --- finished mask --------------------------------------------
            gt = mpool.tile([128, BC // 128], I32, tag="grp")
            nc.sync.dma_start(
                out=gt[:], in_=grp_d.rearrange("(p j) -> p j", p=128)
            )
            ft_ = mpool.tile([128, BC // 128], I32, tag="fin")
            nc.vector.tensor_scalar(ft_[:], gt[:], 2, None,
                                    op0=mybir.AluOpType.is_equal)
            nc.gpsimd.dma_start(
                out=fin_d.rearrange("(p j) -> p j", p=128), in_=ft_[:]
            )

            x1t_r = x1t_d.rearrange("(kt p) n -> p kt n", p=128)
            lt_r = lt_d.rearrange("(kt p) n -> p kt n", p=128)
            outt_r = outt_d.rearrange("(ft p) n -> p ft n", p=128)

            def branch(col0, cap, w1h, w1g, b1h, b1g, w2h, w2g, b2h, b2g,
                       with_lt):
                c0 = 0
                for rblk in _blocks(cap):
                    x_sb = xpool.tile([128, 12, rblk], BF16, tag="x")
                    nc.sync.dma_start(
                        out=x_sb[:],
                        in_=x1t_r[:, :, col0 + c0: col0 + c0 + rblk],
                    )
                    if with_lt:
                        lt_sb = xpool.tile([128, 4, rblk], BF16, tag="ltx")
                        nc.sync.dma_start(
                            out=lt_sb[:],
                            in_=lt_r[:, :, c0: c0 + rblk],
                        )
                    h_sb = hpool.tile([128, 4, rblk], BF16, tag="h")
                    for ft in range(4):
                        ph = pspool.tile([128, rblk], F32, tag="ps")
                        for kt in range(12):
                            nc.tensor.matmul(
                                ph[:],
                                w1h[:, kt, ft * 128:(ft + 1) * 128],
                                x_sb[:, kt, :],
                                start=(kt == 0), stop=(kt == 11),
                            )
                        pg = pspool.tile([128, rblk], F32, tag="ps")
                        for kt in range(12):
                            nc.tensor.matmul(
                                pg[:],
                                w1g[:, kt, ft * 128:(ft + 1) * 128],
                                x_sb[:, kt, :],
                                start=(kt == 0), stop=(kt == 11),
                            )
                        th = apool.tile([128, rblk], BF16, tag="th")
                        sg = apool.tile([128, rblk], BF16, tag="sg")
                        nc.scalar.activation(
                            th[:], ph[:], mybir.ActivationFunctionType.Tanh,
                            bias=b1h[:, ft:ft + 1])
                        nc.scalar.activation(
                            sg[:], pg[:], mybir.ActivationFunctionType.Sigmoid,
                            bias=b1g[:, ft:ft + 1])
                        nc.vector.tensor_mul(h_sb[:, ft, :], th[:], sg[:])

                    out_sb = opool.tile([128, 4, rblk], F32, tag="o")
                    nk2 = 8 if with_lt else 4
                    for ft in range(4):
                        ph = pspool.tile([128, rblk], F32, tag="ps")
                        for kt in range(nk2):
                            rhs = (h_sb[:, kt, :] if kt < 4
                                   else lt_sb[:, kt - 4, :])
                            nc.tensor.matmul(
                                ph[:],
                                w2h[:, kt, ft * 128:(ft + 1) * 128],
                                rhs,
                                start=(kt == 0), stop=(kt == nk2 - 1),
                            )
                        pg = pspool.tile([128, rblk], F32, tag="ps")
                        for kt in range(nk2):
                            rhs = (h_sb[:, kt, :] if kt < 4
                                   else lt_sb[:, kt - 4, :])
                            nc.tensor.matmul(
                                pg[:],
                                w2g[:, kt, ft * 128:(ft + 1) * 128],
                                rhs,
                                start=(kt == 0), stop=(kt == nk2 - 1),
                            )
                        th = apool.tile([128, rblk], F32, tag="th2")
                        sg = apool.tile([128, rblk], F32, tag="sg2")
                        nc.scalar.activation(
                            th[:], ph[:], mybir.ActivationFunctionType.Tanh,
                            bias=b2h[:, ft:ft + 1])
                        nc.scalar.activation(
                            sg[:], pg[:], mybir.ActivationFunctionType.Sigmoid,
                            bias=b2g[:, ft:ft + 1])
                        nc.vector.tensor_mul(out_sb[:, ft, :], th[:], sg[:])
                    nc.gpsimd.dma_start(
                        out=outt_r[:, :, col0 + c0: col0 + c0 + rblk],
                        in_=out_sb[:],
                    )
                    c0 += rblk

            branch(0, cap0,
                   w_sb["wl1h"], w_sb["wl1g"], b_sb["bl1h"], b_sb["bl1g"],
                   w_sb["wl2h"], w_sb["wl2g"], b_sb["bl2h"], b_sb["bl2g"],
                   with_lt=False)
            branch(cap0, cap1,
                   w_sb["wr1h"], w_sb["wr1g"], b_sb["br1h"], b_sb["br1g"],
                   w_sb["wr2h"], w_sb["wr2g"], b_sb["br2h"], b_sb["br2g"],
                   with_lt=True)

    nc.compile()
    return nc


def _get_program(cap0, cap1):
    key = (cap0, cap1)
    if key not in _PROGRAM_CACHE:
        _PROGRAM_CACHE[key] = build_program(cap0, cap1)
    return _PROGRAM_CACHE[key]


def _roundup(x, m):
    return ((x + m - 1) // m) * m


def kernel(node_hidden, node_context, label_embedding, left_embedding, group,
           Wl1h, bl1h, Wl1g, bl1g, Wl2h, bl2h, Wl2g, bl2g,
           Wr1h, br1h, Wr1g, br1g, Wr2h, br2h, Wr2g, br2g,
           trace=False, trace_kwargs=None):
    global LAST_RESULT
    group = np.asarray(group)
    idx0 = np.flatnonzero(group == 0)
    idx1 = np.flatnonzero(group == 1)
    per0 = [idx0[c::NCORES] for c in range(NCORES)]
    per1 = [idx1[c::NCORES] for c in range(NCORES)]
    need0 = max(len(p) for p in per0)
    need1 = max(len(p) for p in per1)
    cap0 = DEFAULT_CAP if need0 <= DEFAULT_CAP else _roundup(need0, 64)
    cap1 = DEFAULT_CAP if need1 <= DEFAULT_CAP else _roundup(need1, 64)

    nc = _get_program(cap0, cap1)

    bf = ml_dtypes.bfloat16
    xcat = np.concatenate(
        [np.asarray(node_hidden), np.asarray(node_context),
         np.asarray(label_embedding)], axis=1)  # [B, 3H] f32
    lemb = np.asarray(left_embedding)

    shared = {
        "wl1h": np.ascontiguousarray(Wl1h).astype(bf),
        "wl1g": np.ascontiguousarray(Wl1g).astype(bf),
        "wr1h": np.ascontiguousarray(Wr1h).astype(bf),
        "wr1g": np.ascontiguousarray(Wr1g).astype(bf),
        "wl2h": np.ascontiguousarray(Wl2h).astype(bf),
        "wl2g": np.ascontiguousarray(Wl2g).astype(bf),
        "wr2h": np.ascontiguousarray(Wr2h).astype(bf),
        "wr2g": np.ascontiguousarray(Wr2g).astype(bf),
        "bl1h": np.ascontiguousarray(bl1h, dtype=np.float32),
        "bl1g": np.ascontiguousarray(bl1g, dtype=np.float32),
        "bl2h": np.ascontiguousarray(bl2h, dtype=np.float32),
        "bl2g": np.ascontiguousarray(bl2g, dtype=np.float32),
        "br1h": np.ascontiguousarray(br1h, dtype=np.float32),
        "br1g": np.ascontiguousarray(br1g, dtype=np.float32),
        "br2h": np.ascontiguousarray(br2h, dtype=np.float32),
        "br2g": np.ascontiguousarray(br2g, dtype=np.float32),
    }

    in_maps = []
    for c in range(NCORES):
        rows0 = np.zeros(cap0, dtype=np.int64)
        rows0[:len(per0[c])] = per0[c]
        rows1 = np.zeros(cap1, dtype=np.int64)
        rows1[:len(per1[c])] = per1[c]
        rows = np.concatenate([rows0, rows1])
        x1t = np.ascontiguousarray(xcat[rows].T).astype(bf)  # [3H, ncols]
        lt = np.ascontiguousarray(lemb[rows1].T).astype(bf)  # [H, cap1]
        m = dict(shared)
        m["x1t"] = x1t
        m["lt"] = lt
        m["grp"] = np.ascontiguousarray(group[c * BC:(c + 1) * BC],
                                        dtype=np.int32)
        in_maps.append(m)

    res = run_bass_kernel_spmd(nc, in_maps, list(range(NCORES)),
                               trace=trace, **(trace_kwargs or {}))
    LAST_RESULT = res

    children = np.zeros((B, H), dtype=np.float32)
    finished = np.empty(B, dtype=np.int32)
    for c in range(NCORES):
        outt = res.results[c]["outt"]  # [H, ncols] f32
        outr = outt.T  # [ncols, H]
        if len(per0[c]):
            children[per0[c]] = outr[:len(per0[c])]
        if len(per1[c]):
            children[per1[c]] = outr[cap0:cap0 + len(per1[c])]
        finished[c * BC:(c + 1) * BC] = res.results[c]["fin"]
    return children, finished


# revision 3
# speedup vs baseline: 1.1251x; 1.1251x over previous
"""Trainium2 Bass kernel for nn_DecomposeModel (gated 2-layer MLP decompose).

Strategy:
  - Host: sort rows by group. Only group==0 rows need the left GateNN,
    only group==1 rows need the right GateNN, group==2 rows output zero.
    Deal g0/g1 rows round-robin across the 8 cores (data parallel), pad
    each segment to a fixed per-core cap so all cores run one SPMD program.
  - Device: activations kept transposed [feature, row] so every matmul
    uses the weights in native [in, out] layout as the stationary operand
    (out = W_tile.T @ xT_tile). bf16 matmuls, f32 PSUM accumulation,
    tanh/sigmoid on ScalarE with fused bias, gating product on VectorE.
    Input x-stream DMAs ride the Sync HWDGE ring, weights ride the
    Scalar HWDGE ring, outputs ride GpSimd SWDGE — three independent
    issue paths so the weight preload doesn't stall the first blocks.
  - Host: scatter device outputs back to full [B, H] f32 (g2 rows stay 0).
"""

import sys

try:
    import concourse  # noqa: F401
except ImportError:
    sys.path.insert(0, "/opt/trn_rl_repo")

import numpy as np
import ml_dtypes

import concourse.tile as tile
from concourse import bacc, mybir
from concourse.bass_utils import run_bass_kernel_spmd

B = 32768
H = 512
NCORES = 8
BC = B // NCORES  # per-core shard of the mask output
DEFAULT_CAP = 1408  # per-core per-branch row capacity (B/3/8 = 1365.3 avg)

BF16 = mybir.dt.bfloat16
F32 = mybir.dt.float32
I32 = mybir.dt.int32

# biases stacked [8, 512] in this order
BIAS_ORDER = ["bl1h", "bl1g", "bl2h", "bl2g", "br1h", "br1g", "br2h", "br2g"]

_PROGRAM_CACHE = {}
LAST_RESULT = None  # BassKernelResults of the most recent kernel() call


def _blocks(cap):
    out = []
    rem = cap
    while rem > 0:
        b = 512 if rem >= 576 else rem
        out.append(b)
        rem -= b
    return out


def build_program(cap0, cap1):
    """Emit + compile the SPMD program for per-branch caps (cap0, cap1)."""
    nc = bacc.Bacc("TRN2", target_bir_lowering=False, debug=False,
                   num_devices=NCORES)

    ncols = cap0 + cap1
    x1t_d = nc.dram_tensor("x1t", [3 * H, ncols], BF16, kind="ExternalInput")
    lt_d = nc.dram_tensor("lt", [H, cap1], BF16, kind="ExternalInput")
    grp_d = nc.dram_tensor("grp", [BC], I32, kind="ExternalInput")
    bias_d = nc.dram_tensor("bias8", [8, H], F32, kind="ExternalInput")

    w_shapes = [("wl1h", 3 * H), ("wl1g", 3 * H),
                ("wl2h", H), ("wl2g", H),
                ("wr1h", 3 * H), ("wr1g", 3 * H),
                ("wr2h", 2 * H), ("wr2g", 2 * H)]
    w_d = {}
    for name, kdim in w_shapes:
        w_d[name] = nc.dram_tensor(name, [kdim, H], BF16, kind="ExternalInput")

    outt_d = nc.dram_tensor("outt", [H, ncols], F32, kind="ExternalOutput")
    fin_d = nc.dram_tensor("fin", [BC], I32, kind="ExternalOutput")

    with tile.TileContext(nc) as tc:
        with (
            tc.tile_pool(name="wsb", bufs=1) as wpool,
            tc.tile_pool(name="bsb", bufs=1) as bpool,
            tc.tile_pool(name="xsb", bufs=3) as xpool,
            tc.tile_pool(name="hsb", bufs=2) as hpool,
            tc.tile_pool(name="act", bufs=3) as apool,
            tc.tile_pool(name="osb", bufs=6) as opool,
            tc.tile_pool(name="msc", bufs=1) as mpool,
            tc.tile_pool(name="ps", bufs=8, space="PSUM") as pspool,
        ):
            # --- persistent weights / biases (Scalar HWDGE ring) -----------
            b_sb = bpool.tile([128, 8, 4], F32, tag="bias8")
            nc.scalar.dma_start(
                out=b_sb[:], in_=bias_d.rearrange("b (j p) -> p b j", p=128)
            )
            bias_ap = {n: b_sb[:, i, :] for i, n in enumerate(BIAS_ORDER)}

            w_sb = {}
            for name, kdim in w_shapes:
                nk = kdim // 128
                t = wpool.tile([128, nk, H], BF16, tag=f"w_{name}")
                nc.scalar.dma_start(
                    out=t[:],
                    in_=w_d[name].rearrange("(kt p) f -> p kt f", p=128),
                )
                w_sb[name] = t

            # --- finished mask (GpSimd SWDGE) ------------------------------
            gt = mpool.tile([128, BC // 128], I32, tag="grp")
            nc.gpsimd.dma_start(
                out=gt[:], in_=grp_d.rearrange("(p j) -> p j", p=128)
            )
            ft_ = mpool.tile([128, BC // 128], I32, tag="fin")
            nc.vector.tensor_scalar(ft_[:], gt[:], 2, None,
                                    op0=mybir.AluOpType.is_equal)
            nc.gpsimd.dma_start(
                out=fin_d.rearrange("(p j) -> p j", p=128), in_=ft_[:]
            )

            x1t_r = x1t_d.rearrange("(kt p) n -> p kt n", p=128)
            lt_r = lt_d.rearrange("(kt p) n -> p kt n", p=128)
            outt_r = outt_d.rearrange("(ft p) n -> p ft n", p=128)

            def branch(col0, cap, w1h, w1g, b1h, b1g, w2h, w2g, b2h, b2g,
                       with_lt):
                c0 = 0
                for rblk in _blocks(cap):
                    x_sb = xpool.tile([128, 12, rblk], BF16, tag="x")
                    nc.sync.dma_start(
                        out=x_sb[:],
                        in_=x1t_r[:, :, col0 + c0: col0 + c0 + rblk],
                    )
                    if with_lt:
                        lt_sb = xpool.tile([128, 4, rblk], BF16, tag="ltx")
                        nc.sync.dma_start(
                            out=lt_sb[:],
                            in_=lt_r[:, :, c0: c0 + rblk],
                        )
                    h_sb = hpool.tile([128, 4, rblk], BF16, tag="h")
                    for ft in range(4):
                        ph = pspool.tile([128, rblk], F32, tag="ps")
                        for kt in range(12):
                            nc.tensor.matmul(
                                ph[:],
                                w1h[:, kt, ft * 128:(ft + 1) * 128],
                                x_sb[:, kt, :],
                                start=(kt == 0), stop=(kt == 11),
                            )
                        pg = pspool.tile([128, rblk], F32, tag="ps")
                        for kt in range(12):
                            nc.tensor.matmul(
                                pg[:],
                                w1g[:, kt, ft * 128:(ft + 1) * 128],
                                x_sb[:, kt, :],
                                start=(kt == 0), stop=(kt == 11),
                            )
                        th = apool.tile([128, rblk], BF16, tag="th")
                        sg = apool.tile([128, rblk], BF16, tag="sg")
                        nc.scalar.activation(
                            th[:], ph[:], mybir.ActivationFunctionType.Tanh,
                            bias=b1h[:, ft:ft + 1])
                        nc.scalar.activation(
                            sg[:], pg[:], mybir.ActivationFunctionType.Sigmoid,
                            bias=b1g[:, ft:ft + 1])
                        nc.vector.tensor_mul(h_sb[:, ft, :], th[:], sg[:])

                    nk2 = 8 if with_lt else 4
                    for ft in range(4):
                        ph = pspool.tile([128, rblk], F32, tag="ps")
                        for kt in range(nk2):
                            rhs = (h_sb[:, kt, :] if kt < 4
                                   else lt_sb[:, kt - 4, :])
                            nc.tensor.matmul(
                                ph[:],
                                w2h[:, kt, ft * 128:(ft + 1) * 128],
                                rhs,
                                start=(kt == 0), stop=(kt == nk2 - 1),
                            )
                        pg = pspool.tile([128, rblk], F32, tag="ps")
                        for kt in range(nk2):
                            rhs = (h_sb[:, kt, :] if kt < 4
                                   else lt_sb[:, kt - 4, :])
                            nc.tensor.matmul(
                                pg[:],
                                w2g[:, kt, ft * 128:(ft + 1) * 128],
                                rhs,
                                start=(kt == 0), stop=(kt == nk2 - 1),
                            )
                        th = apool.tile([128, rblk], F32, tag="th2")
                        sg = apool.tile([128, rblk], F32, tag="sg2")
                        nc.scalar.activation(
                            th[:], ph[:], mybir.ActivationFunctionType.Tanh,
                            bias=b2h[:, ft:ft + 1])
                        nc.scalar.activation(
                            sg[:], pg[:], mybir.ActivationFunctionType.Sigmoid,
                            bias=b2g[:, ft:ft + 1])
                        o_sb = opool.tile([128, rblk], F32, tag="o")
                        nc.vector.tensor_mul(o_sb[:], th[:], sg[:])
                        nc.gpsimd.dma_start(
                            out=outt_r[:, ft, col0 + c0: col0 + c0 + rblk],
                            in_=o_sb[:],
                        )
                    c0 += rblk

            branch(0, cap0,
                   w_sb["wl1h"], w_sb["wl1g"], bias_ap["bl1h"], bias_ap["bl1g"],
                   w_sb["wl2h"], w_sb["wl2g"], bias_ap["bl2h"], bias_ap["bl2g"],
                   with_lt=False)
            branch(cap0, cap1,
                   w_sb["wr1h"], w_sb["wr1g"], bias_ap["br1h"], bias_ap["br1g"],
                   w_sb["wr2h"], w_sb["wr2g"], bias_ap["br2h"], bias_ap["br2g"],
                   with_lt=True)

    nc.compile()
    return nc


def _get_program(cap0, cap1):
    key = (cap0, cap1)
    if key not in _PROGRAM_CACHE:
        _PROGRAM_CACHE[key] = build_program(cap0, cap1)
    return _PROGRAM_CACHE[key]


def _roundup(x, m):
    return ((x + m - 1) // m) * m


def kernel(node_hidden, node_context, label_embedding, left_embedding, group,
           Wl1h, bl1h, Wl1g, bl1g, Wl2h, bl2h, Wl2g, bl2g,
           Wr1h, br1h, Wr1g, br1g, Wr2h, br2h, Wr2g, br2g,
           trace=False, trace_kwargs=None):
    global LAST_RESULT
    group = np.asarray(group)
    idx0 = np.flatnonzero(group == 0)
    idx1 = np.flatnonzero(group == 1)
    per0 = [idx0[c::NCORES] for c in range(NCORES)]
    per1 = [idx1[c::NCORES] for c in range(NCORES)]
    need0 = max(len(p) for p in per0)
    need1 = max(len(p) for p in per1)
    cap0 = DEFAULT_CAP if need0 <= DEFAULT_CAP else _roundup(need0, 64)
    cap1 = DEFAULT_CAP if need1 <= DEFAULT_CAP else _roundup(need1, 64)

    nc = _get_program(cap0, cap1)

    bf = ml_dtypes.bfloat16
    xcat = np.concatenate(
        [np.asarray(node_hidden), np.asarray(node_context),
         np.asarray(label_embedding)], axis=1)  # [B, 3H] f32
    lemb = np.asarray(left_embedding)

    shared = {
        "wl1h": np.ascontiguousarray(Wl1h).astype(bf),
        "wl1g": np.ascontiguousarray(Wl1g).astype(bf),
        "wr1h": np.ascontiguousarray(Wr1h).astype(bf),
        "wr1g": np.ascontiguousarray(Wr1g).astype(bf),
        "wl2h": np.ascontiguousarray(Wl2h).astype(bf),
        "wl2g": np.ascontiguousarray(Wl2g).astype(bf),
        "wr2h": np.ascontiguousarray(Wr2h).astype(bf),
        "wr2g": np.ascontiguousarray(Wr2g).astype(bf),
        "bias8": np.ascontiguousarray(np.stack(
            [bl1h, bl1g, bl2h, bl2g, br1h, br1g, br2h, br2g]),
            dtype=np.float32),
    }

    in_maps = []
    for c in range(NCORES):
        rows0 = np.zeros(cap0, dtype=np.int64)
        rows0[:len(per0[c])] = per0[c]
        rows1 = np.zeros(cap1, dtype=np.int64)
        rows1[:len(per1[c])] = per1[c]
        rows = np.concatenate([rows0, rows1])
        x1t = np.ascontiguousarray(xcat[rows].T).astype(bf)  # [3H, ncols]
        lt = np.ascontiguousarray(lemb[rows1].T).astype(bf)  # [H, cap1]
        m = dict(shared)
        m["x1t"] = x1t
        m["lt"] = lt
        m["grp"] = np.ascontiguousarray(group[c * BC:(c + 1) * BC],
                                        dtype=np.int32)
        in_maps.append(m)

    res = run_bass_kernel_spmd(nc, in_maps, list(range(NCORES)),
                               trace=trace, **(trace_kwargs or {}))
    LAST_RESULT = res

    children = np.zeros((B, H), dtype=np.float32)
    finished = np.empty(B, dtype=np.int32)
    for c in range(NCORES):
        outt = res.results[c]["outt"]  # [H, ncols] f32
        outr = outt.T  # [ncols, H]
        if len(per0[c]):
            children[per0[c]] = outr[:len(per0[c])]
        if len(per1[c]):
            children[per1[c]] = outr[cap0:cap0 + len(per1[c])]
        finished[c * BC:(c + 1) * BC] = res.results[c]["fin"]
    return children, finished


# revision 9
# speedup vs baseline: 1.1961x; 1.0631x over previous
"""Trainium2 Bass kernel for nn_DecomposeModel (gated 2-layer MLP decompose).

Strategy:
  - Host: sort rows by group. Only group==0 rows need the left GateNN,
    only group==1 rows need the right GateNN, group==2 rows output zero.
    Deal g0/g1 rows round-robin across the 8 cores (data parallel), pad
    each segment to a fixed per-core cap so all cores run one SPMD program.
  - Device: activations kept transposed [feature, row] so every matmul
    uses the weights in native [in, out] layout as the stationary operand
    (out = W_tile.T @ xT_tile). bf16 matmuls, f32 PSUM accumulation,
    tanh/sigmoid on ScalarE with fused bias, gating product on VectorE.
    Input x-stream DMAs ride the Sync HWDGE ring, weights ride the
    Scalar HWDGE ring, outputs ride GpSimd SWDGE — three independent
    issue paths so the weight preload doesn't stall the first blocks.
  - Host: scatter device outputs back to full [B, H] f32 (g2 rows stay 0).
"""

import sys

try:
    import concourse  # noqa: F401
except ImportError:
    sys.path.insert(0, "/opt/trn_rl_repo")

import numpy as np
import ml_dtypes

import concourse.tile as tile
from concourse import bacc, mybir
from concourse.bass_utils import run_bass_kernel_spmd

B = 32768
H = 512
NCORES = 8
BC = B // NCORES  # per-core shard of the mask output
DEFAULT_CAP = 1408  # per-core per-branch row capacity (B/3/8 = 1365.3 avg)

BF16 = mybir.dt.bfloat16
F32 = mybir.dt.float32
I32 = mybir.dt.int32

# biases stacked [8, 512] in this order
BIAS_ORDER = ["bl1h", "bl1g", "bl2h", "bl2g", "br1h", "br1g", "br2h", "br2g"]

_PROGRAM_CACHE = {}
LAST_RESULT = None  # BassKernelResults of the most recent kernel() call


def _blocks(cap):
    out = []
    rem = cap
    while rem > 0:
        b = 512 if rem >= 576 else rem
        out.append(b)
        rem -= b
    return out


def build_program(cap0, cap1):
    """Emit + compile the SPMD program for per-branch caps (cap0, cap1)."""
    nc = bacc.Bacc("TRN2", target_bir_lowering=False, debug=False,
                   num_devices=NCORES)

    ncols = cap0 + cap1
    x1t_d = nc.dram_tensor("x1t", [3 * H, ncols], BF16, kind="ExternalInput")
    lt_d = nc.dram_tensor("lt", [H, cap1], BF16, kind="ExternalInput")
    grp_d = nc.dram_tensor("grp", [BC], I32, kind="ExternalInput")
    bias_d = nc.dram_tensor("bias8", [8, H], F32, kind="ExternalInput")

    w_shapes = [("wl1h", 3 * H), ("wl1g", 3 * H),
                ("wl2h", H), ("wl2g", H),
                ("wr1h", 3 * H), ("wr1g", 3 * H),
                ("wr2h", 2 * H), ("wr2g", 2 * H)]
    w_d = {}
    for name, kdim in w_shapes:
        w_d[name] = nc.dram_tensor(name, [kdim, H], BF16, kind="ExternalInput")

    outt_d = nc.dram_tensor("outt", [H, ncols], F32, kind="ExternalOutput")
    fin_d = nc.dram_tensor("fin", [BC], I32, kind="ExternalOutput")

    with tile.TileContext(nc) as tc:
        with (
            tc.tile_pool(name="wsb", bufs=1) as wpool,
            tc.tile_pool(name="bsb", bufs=1) as bpool,
            tc.tile_pool(name="xsb", bufs=3) as xpool,
            tc.tile_pool(name="hsb", bufs=2) as hpool,
            tc.tile_pool(name="act", bufs=3) as apool,
            tc.tile_pool(name="osb", bufs=6) as opool,
            tc.tile_pool(name="msc", bufs=1) as mpool,
            tc.tile_pool(name="ps", bufs=8, space="PSUM") as pspool,
        ):
            # --- persistent weights / biases (Sync HWDGE ring) -------------
            # One FIFO ring, hand-ordered: the first matmul only gates on
            # bias8 + wl1h + x-block-0; remaining weight loads are emitted
            # interleaved after later x-block DMAs (see _pending_w below).
            b_sb = bpool.tile([128, 8, 4], F32, tag="bias8")
            nc.sync.dma_start(
                out=b_sb[:], in_=bias_d.rearrange("b (j p) -> p b j", p=128)
            )
            bias_ap = {n: b_sb[:, i, :] for i, n in enumerate(BIAS_ORDER)}

            w_sb = {}
            for name, kdim in w_shapes:
                nk = kdim // 128
                t = wpool.tile([128, nk, H], BF16, tag=f"w_{name}")
                w_sb[name] = t

            def _load_w(name):
                nc.sync.dma_start(
                    out=w_sb[name][:],
                    in_=w_d[name].rearrange("(kt p) f -> p kt f", p=128),
                )

            _load_w("wl1h")

            # --- finished mask (GpSimd SWDGE) ------------------------------
            gt = mpool.tile([128, BC // 128], I32, tag="grp")
            nc.gpsimd.dma_start(
                out=gt[:], in_=grp_d.rearrange("(p j) -> p j", p=128)
            )
            ft_ = mpool.tile([128, BC // 128], I32, tag="fin")
            nc.vector.tensor_scalar(ft_[:], gt[:], 2, None,
                                    op0=mybir.AluOpType.is_equal)
            nc.gpsimd.dma_start(
                out=fin_d.rearrange("(p j) -> p j", p=128), in_=ft_[:]
            )

            x1t_r = x1t_d.rearrange("(kt p) n -> p kt n", p=128)
            lt_r = lt_d.rearrange("(kt p) n -> p kt n", p=128)
            outt_r = outt_d.rearrange("(ft p) n -> p ft n", p=128)

            def branch(col0, cap, w1h, w1g, b1h, b1g, w2h, w2g, b2h, b2g,
                       with_lt, deferred_w=()):
                deferred_w = list(deferred_w)
                c0 = 0
                for rblk in _blocks(cap):
                    x_sb = xpool.tile([128, 12, rblk], BF16, tag="x")
                    nc.sync.dma_start(
                        out=x_sb[:],
                        in_=x1t_r[:, :, col0 + c0: col0 + c0 + rblk],
                    )
                    while deferred_w:
                        _load_w(deferred_w.pop(0))
                    if with_lt:
                        lt_sb = xpool.tile([128, 4, rblk], BF16, tag="ltx")
                        nc.sync.dma_start(
                            out=lt_sb[:],
                            in_=lt_r[:, :, c0: c0 + rblk],
                        )
                    h_sb = hpool.tile([128, 4, rblk], BF16, tag="h")
                    for ft in range(4):
                        ph = pspool.tile([128, rblk], F32, tag="ps")
                        for kt in range(12):
                            nc.tensor.matmul(
                                ph[:],
                                w1h[:, kt, ft * 128:(ft + 1) * 128],
                                x_sb[:, kt, :],
                                start=(kt == 0), stop=(kt == 11),
                            )
                        pg = pspool.tile([128, rblk], F32, tag="ps")
                        for kt in range(12):
                            nc.tensor.matmul(
                                pg[:],
                                w1g[:, kt, ft * 128:(ft + 1) * 128],
                                x_sb[:, kt, :],
                                start=(kt == 0), stop=(kt == 11),
                            )
                        th = apool.tile([128, rblk], BF16, tag="th")
                        sg = apool.tile([128, rblk], BF16, tag="sg")
                        nc.scalar.activation(
                            th[:], ph[:], mybir.ActivationFunctionType.Tanh,
                            bias=b1h[:, ft:ft + 1])
                        nc.scalar.activation(
                            sg[:], pg[:], mybir.ActivationFunctionType.Sigmoid,
                            bias=b1g[:, ft:ft + 1])
                        nc.vector.tensor_mul(h_sb[:, ft, :], th[:], sg[:])

                    nk2 = 8 if with_lt else 4
                    for ft in range(4):
                        ph = pspool.tile([128, rblk], F32, tag="ps")
                        for kt in range(nk2):
                            rhs = (h_sb[:, kt, :] if kt < 4
                                   else lt_sb[:, kt - 4, :])
                            nc.tensor.matmul(
                                ph[:],
                                w2h[:, kt, ft * 128:(ft + 1) * 128],
                                rhs,
                                start=(kt == 0), stop=(kt == nk2 - 1),
                            )
                        pg = pspool.tile([128, rblk], F32, tag="ps")
                        for kt in range(nk2):
                            rhs = (h_sb[:, kt, :] if kt < 4
                                   else lt_sb[:, kt - 4, :])
                            nc.tensor.matmul(
                                pg[:],
                                w2g[:, kt, ft * 128:(ft + 1) * 128],
                                rhs,
                                start=(kt == 0), stop=(kt == nk2 - 1),
                            )
                        th = apool.tile([128, rblk], F32, tag="th2")
                        sg = apool.tile([128, rblk], F32, tag="sg2")
                        nc.scalar.activation(
                            th[:], ph[:], mybir.ActivationFunctionType.Tanh,
                            bias=b2h[:, ft:ft + 1])
                        nc.scalar.activation(
                            sg[:], pg[:], mybir.ActivationFunctionType.Sigmoid,
                            bias=b2g[:, ft:ft + 1])
                        o_sb = opool.tile([128, rblk], F32, tag="o")
                        nc.vector.tensor_mul(o_sb[:], th[:], sg[:])
                        nc.gpsimd.dma_start(
                            out=outt_r[:, ft, col0 + c0: col0 + c0 + rblk],
                            in_=o_sb[:],
                        )
                    c0 += rblk

            branch(0, cap0,
                   w_sb["wl1h"], w_sb["wl1g"], bias_ap["bl1h"], bias_ap["bl1g"],
                   w_sb["wl2h"], w_sb["wl2g"], bias_ap["bl2h"], bias_ap["bl2g"],
                   with_lt=False, deferred_w=["wl1g", "wl2h", "wl2g"])
            branch(cap0, cap1,
                   w_sb["wr1h"], w_sb["wr1g"], bias_ap["br1h"], bias_ap["br1g"],
                   w_sb["wr2h"], w_sb["wr2g"], bias_ap["br2h"], bias_ap["br2g"],
                   with_lt=True,
                   deferred_w=["wr1h", "wr1g", "wr2h", "wr2g"])

    nc.compile()
    return nc


def _get_program(cap0, cap1):
    key = (cap0, cap1)
    if key not in _PROGRAM_CACHE:
        _PROGRAM_CACHE[key] = build_program(cap0, cap1)
    return _PROGRAM_CACHE[key]


def _roundup(x, m):
    return ((x + m - 1) // m) * m


def kernel(node_hidden, node_context, label_embedding, left_embedding, group,
           Wl1h, bl1h, Wl1g, bl1g, Wl2h, bl2h, Wl2g, bl2g,
           Wr1h, br1h, Wr1g, br1g, Wr2h, br2h, Wr2g, br2g,
           trace=False, trace_kwargs=None):
    global LAST_RESULT
    group = np.asarray(group)
    idx0 = np.flatnonzero(group == 0)
    idx1 = np.flatnonzero(group == 1)
    per0 = [idx0[c::NCORES] for c in range(NCORES)]
    per1 = [idx1[c::NCORES] for c in range(NCORES)]
    need0 = max(len(p) for p in per0)
    need1 = max(len(p) for p in per1)
    cap0 = DEFAULT_CAP if need0 <= DEFAULT_CAP else _roundup(need0, 64)
    cap1 = DEFAULT_CAP if need1 <= DEFAULT_CAP else _roundup(need1, 64)

    nc = _get_program(cap0, cap1)

    bf = ml_dtypes.bfloat16
    xcat = np.concatenate(
        [np.asarray(node_hidden), np.asarray(node_context),
         np.asarray(label_embedding)], axis=1)  # [B, 3H] f32
    lemb = np.asarray(left_embedding)

    shared = {
        "wl1h": np.ascontiguousarray(Wl1h).astype(bf),
        "wl1g": np.ascontiguousarray(Wl1g).astype(bf),
        "wr1h": np.ascontiguousarray(Wr1h).astype(bf),
        "wr1g": np.ascontiguousarray(Wr1g).astype(bf),
        "wl2h": np.ascontiguousarray(Wl2h).astype(bf),
        "wl2g": np.ascontiguousarray(Wl2g).astype(bf),
        "wr2h": np.ascontiguousarray(Wr2h).astype(bf),
        "wr2g": np.ascontiguousarray(Wr2g).astype(bf),
        "bias8": np.ascontiguousarray(np.stack(
            [bl1h, bl1g, bl2h, bl2g, br1h, br1g, br2h, br2g]),
            dtype=np.float32),
    }

    in_maps = []
    for c in range(NCORES):
        rows0 = np.zeros(cap0, dtype=np.int64)
        rows0[:len(per0[c])] = per0[c]
        rows1 = np.zeros(cap1, dtype=np.int64)
        rows1[:len(per1[c])] = per1[c]
        rows = np.concatenate([rows0, rows1])
        x1t = np.ascontiguousarray(xcat[rows].T).astype(bf)  # [3H, ncols]
        lt = np.ascontiguousarray(lemb[rows1].T).astype(bf)  # [H, cap1]
        m = dict(shared)
        m["x1t"] = x1t
        m["lt"] = lt
        m["grp"] = np.ascontiguousarray(group[c * BC:(c + 1) * BC],
                                        dtype=np.int32)
        in_maps.append(m)

    res = run_bass_kernel_spmd(nc, in_maps, list(range(NCORES)),
                               trace=trace, **(trace_kwargs or {}))
    LAST_RESULT = res

    children = np.zeros((B, H), dtype=np.float32)
    finished = np.empty(B, dtype=np.int32)
    for c in range(NCORES):
        outt = res.results[c]["outt"]  # [H, ncols] f32
        outr = outt.T  # [ncols, H]
        if len(per0[c]):
            children[per0[c]] = outr[:len(per0[c])]
        if len(per1[c]):
            children[per1[c]] = outr[cap0:cap0 + len(per1[c])]
        finished[c * BC:(c + 1) * BC] = res.results[c]["fin"]
    return children, finished
